# revision 42
# baseline (speedup 1.0000x reference)
"""Trainium2 Bass kernel for nn_EfficientSpatioTemporalBlock (v2).

Sharding: 8 cores = (batch 4) x (H halves 2). Per-core shard: one sample,
32 own H rows (+1 halo row each side). All intermediates live in SBUF (bf16).

v2 engine doctrine (from cost-model probing):
  - DVE tensor_scalar (even with per-partition AP scalars) runs 4x for bf16:
    all preps / affines / small copies go there.
  - STT / fp32 / PSUM-source DVE ops are 1x (~1.07 ns/elem).
  - ACT has ~1.4us fixed cost per op: only big-FD PSUM->SBUF copies.
  - POOL (gpsimd) takes sumsq halves and final max/add ops.
  - PE: stage1 matmul, 9 spatial taps, 2 temporal taps (diag), proj.
  - Collectives: AllGather (15us) instead of AllReduce (28us), staggered
    per channel-block so they overlap the other block's compute.
  - No DRAM round-trip for proj output (written in-place into A0),
    bf16 residual + bf16 output (host casts to fp32).
"""

import sys

sys.path.insert(0, "/opt/trn_rl_repo")

import numpy as np
import ml_dtypes

import concourse.bass as bass
import concourse.mybir as mybir
from concourse.tile import TileContext
from concourse.bass_utils import run_bass_kernel_spmd

F32 = mybir.dt.float32
BF16 = mybir.dt.bfloat16
AX = mybir.AxisListType
OP = mybir.AluOpType
AF = mybir.ActivationFunctionType

CIN, HID, CO = 64, 256, 64
T, H, W = 16, 64, 64
YS, YH = 32, 34
NPIX = float(T * H * W)
EPS = 1e-5
DEAD_M = 1e30

# spatial taps excluding center (dy, dx)
TAPS8 = [(dy, dx) for dy in range(3) for dx in range(3) if not (dy == 1 and dx == 1)]


def _build_nc():
    nc = bass.Bass()

    xs16 = nc.declare_dram_parameter("xs16", [CIN, T, YH, W], BF16, isOutput=False)
    w1t = nc.declare_dram_parameter("w1t", [CIN, HID], BF16, isOutput=False)
    diag8 = nc.declare_dram_parameter("diag8", [128, 16 * 128], BF16, isOutput=False)
    wcd = nc.declare_dram_parameter("wcd", [128, 2], F32, isOutput=False)
    wtd = nc.declare_dram_parameter("wtd", [128, 6 * 128], BF16, isOutput=False)
    wt1 = nc.declare_dram_parameter("wt1", [128, 2], F32, isOutput=False)
    wse1t = nc.declare_dram_parameter("wse1t", [128, 128], F32, isOutput=False)
    wse2t = nc.declare_dram_parameter("wse2t", [64, 256], F32, isOutput=False)
    wprojt = nc.declare_dram_parameter("wprojt", [128, 128], F32, isOutput=False)
    hs = nc.declare_dram_parameter("hs", [128, 2], F32, isOutput=False)
    xres = nc.declare_dram_parameter("xres", [128, T, 1024], BF16, isOutput=False)
    out = nc.declare_dram_parameter("out", [128, T, 256], BF16, isOutput=True)

    # collectives: 6 stage-blk stats + SE pool + stats4
    cc_i = [nc.dram_tensor(f"cc{i}i", [128, 2], F32) for i in range(7)]
    cc_o = [nc.dram_tensor(f"cc{i}o", [256, 2], F32) for i in range(7)]
    pl_i = [nc.dram_tensor(f"pl{i}i", [128, 1], F32) for i in range(2)]
    pl_o = [nc.dram_tensor(f"pl{i}o", [256, 1], F32) for i in range(2)]
    c4_i = nc.dram_tensor("c4i", [64, 2], F32)
    c4_o = nc.dram_tensor("c4o", [128, 2], F32)
    GROUPS = [[0, 1], [2, 3], [4, 5], [6, 7]]

    from contextlib import ExitStack
    with ExitStack() as stk:
        sb = lambda *a: stk.enter_context(nc.sbuf_tensor(*a))
        A0 = sb("A0", [128, T, YH, W], BF16)
        A1 = sb("A1", [128, T, YH, W], BF16)
        N0 = sb("N0", [128, YH, 68], BF16)
        N1 = sb("N1", [128, YH, 68], BF16)
        M0 = sb("M0", [128, YS, W], BF16)
        M1 = sb("M1", [128, YS, W], BF16)
        M2 = sb("M2", [128, YS, W], BF16)
        M3 = sb("M3", [128, YS, W], BF16)
        MZ = sb("MZ", [128, YS, W], BF16)
        SC = sb("SC", [128, 384], F32)
        SS = sb("SS", [128, 48], F32)
        w1sb = sb("w1sb", [CIN, HID], BF16)
        diagsb = sb("diagsb", [128, 16 * 128], BF16)
        wcdsb = sb("wcdsb", [128, 2], F32)
        wtdsb = sb("wtdsb", [128, 6 * 128], BF16)
        wt1sb = sb("wt1sb", [128, 2], F32)
        wse1sb = sb("wse1sb", [128, 128], F32)
        wse2sb = sb("wse2sb", [64, 256], F32)
        wprojsb = sb("wprojsb", [128, 128], F32)
        wpb = sb("wpb", [128, 128], BF16)
        hssb = sb("hssb", [128, 2], F32)
        zsb = sb("zsb", [64, 1], F32)
        ccs = sb("ccs", [128, 16], F32)
        ccr = [sb(f"ccr{i}", [128, 4], F32) for i in range(8)]
        SCRD = sb("SCRD", [128, 2048], BF16)
        SCRP = sb("SCRP", [128, 2048], BF16)

        PS = nc.alloc_psum_tensor("PS", [128, 4096], F32)

        tc = stk.enter_context(TileContext(nc))
        xin_pool = stk.enter_context(tc.tile_pool(name="xin", bufs=3))
        fin_pool = stk.enter_context(tc.tile_pool(name="fin", bufs=2))
        A = [A0, A1]
        NR = [N0, N1]
        MR = [M0, M1, M2, M3]

        # SS columns (per blk offset b = 16*blk)
        M1C, R1C, M2C, R2C, M3C, R3C = 0, 1, 2, 3, 4, 5
        WC1, WT1C, YA3, POOLC = 6, 7, 8, 9
        TP0, TP1 = 11, 12
        # shared columns
        M4C, R4C, S1F, TPS = 32, 33, 34, 35
        EPSC, ZEROC = 36, 37

        def ss(col, p0=0, p1=128):
            return SS[p0:p1, col:col + 1]

        # ---- load weights ----
        nc.sync.dma_start(out=w1sb[:, :], in_=w1t[:, :])
        nc.sync.dma_start(out=diagsb[:, :], in_=diag8[:, :])
        nc.sync.dma_start(out=wcdsb[:, :], in_=wcd[:, :])
        nc.sync.dma_start(out=wtdsb[:, :], in_=wtd[:, :])
        nc.sync.dma_start(out=wt1sb[:, :], in_=wt1[:, :])
        nc.sync.dma_start(out=wse1sb[:, :], in_=wse1t[:, :])
        nc.sync.dma_start(out=wse2sb[:, :], in_=wse2t[:, :])
        nc.sync.dma_start(out=wprojsb[:, :], in_=wprojt[:, :])
        nc.sync.dma_start(out=hssb[:, :], in_=hs[:, :])
        nc.vector.memset(MZ[:, :, :], 0.0)
        nc.vector.memset(SS[:, :], 0.0)
        nc.vector.memset(SS[:, EPSC:EPSC + 1], EPS)
        for Nt in NR:
            nc.vector.memset(Nt[:, :, 0:2], 0.0)
            nc.vector.memset(Nt[:, :, 66:68], 0.0)

        sc_used = {}

        def sc_col(group, base):
            c = base + sc_used.get(group, 0)
            sc_used[group] = sc_used.get(group, 0) + 1
            return c

        def reduce_cols(dst, group, base, p0=0, p1=128):
            n = sc_used[group]
            nc.vector.tensor_reduce(dst, SC[p0:p1, base:base + n], AX.X, OP.add)

        def stats_from(sum_ap, sq_ap, mcol, rcol, b, p0=0, p1=128):
            # m = S/NPIX ; r = exp(-0.5*ln(S2/NPIX - m^2 + eps))
            nc.vector.tensor_scalar(ss(mcol + b, p0, p1), sum_ap, 1.0 / NPIX, None, OP.mult)
            nc.vector.tensor_scalar(ss(TP0 + b, p0, p1), sq_ap, 1.0 / NPIX, None, OP.mult)
            nc.vector.tensor_tensor(ss(TP1 + b, p0, p1), ss(mcol + b, p0, p1), ss(mcol + b, p0, p1), OP.mult)
            nc.vector.tensor_tensor(ss(TP0 + b, p0, p1), ss(TP0 + b, p0, p1), ss(TP1 + b, p0, p1), OP.subtract)
            nc.vector.tensor_scalar(ss(TP1 + b, p0, p1), ss(TP0 + b, p0, p1),
                                    EPS, None, OP.add)
            nc.vector.reciprocal(ss(TP0 + b, p0, p1), ss(TP1 + b, p0, p1))
            nc.scalar.activation(ss(rcol + b, p0, p1), ss(TP0 + b, p0, p1), AF.Sqrt,
                                 bias=ss(ZEROC, p0, p1), scale=1.0)

        # SC col bases (per stage, per blk): sums and sumsq
        B_S1S = (0, 40)      # 32 chunk cols each
        B_S1Q = (80, 100)    # 16 each
        B_S2S = (120, 140)
        B_S2Q = (160, 180)
        B_S3S = (200, 220)
        B_S3Q = (240, 260)
        B_PL = (280, 300)
        B_S4S = 320
        B_S4Q = 352

        def cc_issue(idx, sgrp, sbase, qgrp, qbase, blk):
            """reduce partial cols -> ccs pair -> DRAM -> AllGather."""
            c0 = 2 * idx
            reduce_cols(ccs[:, c0:c0 + 1], sgrp, sbase)
            reduce_cols(ccs[:, c0 + 1:c0 + 2], qgrp, qbase)
            nc.sync.dma_start(out=cc_i[idx][:, :], in_=ccs[:, c0:c0 + 2])
            nc.gpsimd.collective_compute(
                "AllGather", OP.bypass, replica_groups=GROUPS,
                ins=[cc_i[idx][:, :]], outs=[cc_o[idx][:, :]])

        def cc_finish(idx, mcol, rcol, blk):
            b = 16 * blk
            r = ccr[idx]
            nc.sync.dma_start(
                out=r[:, 0:4].rearrange("p (r c) -> p r c", c=2),
                in_=cc_o[idx][:, :].rearrange("(r p) c -> p r c", p=128))
            nc.vector.tensor_tensor(r[:, 0:2], r[:, 0:2], r[:, 2:4], OP.add)
            stats_from(r[:, 0:1], r[:, 1:2], mcol, rcol, b)

        def fold_r1(blk):
            b = 16 * blk
            nc.vector.tensor_scalar(
                diagsb[:, blk * 1024:(blk + 1) * 1024],
                diagsb[:, blk * 1024:(blk + 1) * 1024], ss(R1C + b), None, OP.mult)
            nc.vector.tensor_tensor(ss(WC1 + b), wcdsb[:, blk:blk + 1], ss(R1C + b), OP.mult)

        def fold_r2(blk):
            b = 16 * blk
            nc.vector.tensor_scalar(
                wtdsb[:, blk * 384:(blk + 1) * 384],
                wtdsb[:, blk * 384:(blk + 1) * 384], ss(R2C + b), None, OP.mult)

        # ================= stage 1: 1x1 conv =================
        # per frame-blk: psum chunks: halo0(64) big0(1024) big1(1024) halo1(64)
        # big-chunk ring of 3 at offsets 0/1024/2048; halo ring of 4 at 3072+
        Y_CHUNKS = [(0, 1), (1, 16), (17, 16), (33, 1)]
        big_i = [0]
        halo_i = [0]

        def s1_frame(blk, f, xt):
            for (y0, rows) in Y_CHUNKS:
                n = rows * W
                if rows == 1:
                    off = 3072 + 64 * (halo_i[0] % 4)
                    halo_i[0] += 1
                else:
                    off = 1024 * (big_i[0] % 3)
                    big_i[0] += 1
                pt = PS[:, off:off + n]
                # matmuls (<=512 cols each)
                for k in range(0, rows, 8):
                    rk = min(8, rows - k)
                    nk = rk * W
                    nc.tensor.matmul(
                        pt[:, k * W:k * W + nk],
                        w1sb[:, blk * 128:(blk + 1) * 128],
                        xt[:, (y0 + k) * W:(y0 + k) * W + nk],
                        start=True, stop=True)
                dst = A[blk][:, f, y0:y0 + rows, :].rearrange("p a b -> p (a b)")
                if rows == 1:
                    nc.vector.tensor_scalar(dst, pt[:, :], 1.0, None, OP.mult)
                else:
                    c = sc_col(("s1s", blk), B_S1S[blk])
                    if f % 3 == 2:
                        nc.scalar.activation(dst, pt[:, :], AF.Copy,
                                             accum_out=SC[:, c:c + 1])
                    else:
                        nc.vector.tensor_scalar(dst, pt[:, :], 1.0, None, OP.mult,
                                                OP.add, accum_out=SC[:, c:c + 1])
            # sumsq over own rows 1:33 (contiguous)
            c = sc_col(("s1q", blk), B_S1Q[blk])
            src = A[blk][:, f, 1:33, :].rearrange("p a b -> p (a b)")
            nc.gpsimd.tensor_tensor(SCRP[:, :], src, src, OP.mult)
            nc.vector.tensor_scalar(SCRD[:, :], SCRP[:, :], 1.0, None,
                                    OP.mult, OP.add, accum_out=SC[:, c:c + 1])

        for blk in range(2):
            for f in range(T):
                xt = xin_pool.tile([CIN, YH * W], BF16, tag="xt")
                nc.sync.dma_start(
                    out=xt[:, :],
                    in_=xs16[:, f, :, :].rearrange("c a b -> c (a b)"))
                s1_frame(blk, f, xt)
                if blk == 1 and f == 8:
                    # stats1(blk0) finish hides under remaining blk1 frames
                    cc_finish(0, M1C, R1C, 0)
                    nc.vector.memset(SS[32:64, M1C:M1C + 1], DEAD_M)
                    fold_r1(0)
            cc_issue(blk, ("s1s", blk), B_S1S[blk], ("s1q", blk), B_S1Q[blk], blk)


        # ================= stage 2: spatial 3x3 (8 taps PE + center fold) ====
        def prep2(blk, f, Nt):
            m1 = ss(M1C + 16 * blk)
            if blk == 0:
                nc.vector.tensor_scalar(
                    Nt[64:128, :, 2:66], A0[64:128, f, :, :],
                    ss(M1C, 64, 128), 0.0, OP.subtract, OP.max)
                if f < T - 1:
                    nc.vector.tensor_scalar(
                        Nt[0:64, :, 2:66], A0[0:64, f + 1, :, :],
                        ss(M1C, 0, 64), 0.0, OP.subtract, OP.max)
                else:
                    nc.vector.tensor_scalar(
                        Nt[0:64, :, 2:66], A0[0:64, f, :, :],
                        0.0, 0.0, OP.mult, OP.mult)
            else:
                nc.vector.tensor_scalar(
                    Nt[:, :, 2:66], A1[:, f, :, :], m1, 0.0, OP.subtract, OP.max)
            nc.vector.tensor_scalar(
                Nt[:, 0, 2:66], Nt[:, 0, 2:66], hssb[:, 0:1], None, OP.mult)
            nc.vector.tensor_scalar(
                Nt[:, 33, 2:66], Nt[:, 33, 2:66], hssb[:, 1:2], None, OP.mult)

        def s2_frame(blk, f, fb):
            Nt = NR[f % 2]
            prep2(blk, f, Nt)
            off = 2048 * (fb % 2)
            for ti, y0 in enumerate((0, 8, 16, 24)):
                pt = PS[:, off + ti * 512: off + (ti + 1) * 512]
                for k, (dy, dx) in enumerate(TAPS8):
                    nc.tensor.matmul(
                        pt[:, :],
                        diagsb[:, (blk * 8 + k) * 128:(blk * 8 + k + 1) * 128],
                        Nt[:, y0 + dy:y0 + dy + 8, 1 + dx:65 + dx],
                        start=(k == 0), stop=(k == 7))
            # combine: A[f] = center*u1 + psum ; accum sum(g2)
            c = sc_col(("s2s", blk), B_S2S[blk])
            nc.vector.scalar_tensor_tensor(
                A[blk][:, f, 0:32, :],
                Nt[:, 1:33, 2:66],
                ss(WC1 + 16 * blk),
                PS[:, off:off + 2048].rearrange("p (a b) -> p a b", b=64),
                OP.mult, OP.add, accum_out=SC[:, c:c + 1])
            # sumsq
            c = sc_col(("s2q", blk), B_S2Q[blk])
            src = A[blk][:, f, 0:32, :].rearrange("p a b -> p (a b)")
            nc.gpsimd.tensor_tensor(SCRP[:, :], src, src, OP.mult)
            nc.vector.tensor_scalar(SCRD[:, :], SCRP[:, :], 1.0, None,
                                    OP.mult, OP.add, accum_out=SC[:, c:c + 1])

        fb = [0]

        def s2_cc(blk):
            cc_issue(2 + blk, ("s2s", blk), B_S2S[blk], ("s2q", blk), B_S2Q[blk], blk)

        # ================= stage 3: temporal 3-tap (2 taps PE + center) =====
        def prep3(blk, f):
            nc.vector.tensor_scalar(
                MR[f % 4][:, :, :], A[blk][:, f, 0:32, :],
                ss(M2C + 16 * blk), 0.0, OP.subtract, OP.max)

        def s3_frame(blk, g, fb):
            b = 16 * blk
            mprev = MZ if g == 0 else MR[(g - 1) % 4]
            mnext = MZ if g == T - 1 else MR[(g + 1) % 4]
            off = 2048 * (fb % 2)
            for ti, y0 in enumerate((0, 8, 16, 24)):
                pt = PS[:, off + ti * 512: off + (ti + 1) * 512]
                nc.tensor.matmul(
                    pt[:, :], wtdsb[:, (blk * 3) * 128:(blk * 3 + 1) * 128],
                    mprev[:, y0:y0 + 8, :], start=True, stop=False)
                nc.tensor.matmul(
                    pt[:, :], wtdsb[:, (blk * 3 + 1) * 128:(blk * 3 + 2) * 128],
                    MR[g % 4][:, y0:y0 + 8, :], start=False, stop=False)
                nc.tensor.matmul(
                    pt[:, :], wtdsb[:, (blk * 3 + 2) * 128:(blk * 3 + 3) * 128],
                    mnext[:, y0:y0 + 8, :], start=False, stop=True)
            c = sc_col(("s3s", blk), B_S3S[blk])
            dst = A[blk][:, g, 0:32, :].rearrange("p a b -> p (a b)")
            if g % 2 == 1:
                nc.scalar.activation(dst, PS[:, off:off + 2048], AF.Copy,
                                     accum_out=SC[:, c:c + 1])
            else:
                nc.vector.tensor_scalar(dst, PS[:, off:off + 2048], 1.0, None,
                                        OP.mult, OP.add, accum_out=SC[:, c:c + 1])
            c = sc_col(("s3q", blk), B_S3Q[blk])
            src = A[blk][:, g, 0:32, :]
            nc.gpsimd.tensor_tensor(
                SCRP[:, :].rearrange("p (a b) -> p a b", b=64), src, src, OP.mult)
            nc.vector.tensor_scalar(SCRD[:, :], SCRP[:, :], 1.0, None,
                                    OP.mult, OP.add, accum_out=SC[:, c:c + 1])

        def s3_cc(blk):
            cc_issue(4 + blk, ("s3s", blk), B_S3S[blk], ("s3q", blk), B_S3Q[blk], blk)

        def se_frame(blk, f):
            b = 16 * blk
            if True:
                c = sc_col(("pl", blk), B_PL[blk])
                ap = A[blk][:, f, 0:32, :].rearrange("p a b -> p (a b)")
                nc.vector.tensor_scalar(
                    ap, ap, ss(M3C + b), 0.0, OP.subtract, OP.max)
                nc.vector.tensor_scalar(
                    SCRD[:, :], ap, 1.0, None,
                    OP.mult, OP.add, accum_out=SC[:, c:c + 1])

        # ---- staggered + interleaved schedule ----
        LEAD = 4
        for f in range(T):
            s2_frame(0, f, fb[0]); fb[0] += 1
            if f == 6:
                cc_finish(1, M1C, R1C, 1)
                fold_r1(1)
        s2_cc(0)
        for f in range(LEAD):
            s2_frame(1, f, fb[0]); fb[0] += 1
            if f == 2:
                cc_finish(2, M2C, R2C, 0)
                fold_r2(0)
        prep3(0, 0)
        g = 0
        for f in range(LEAD, T):
            s2_frame(1, f, fb[0]); fb[0] += 1
            if g < T:
                if g < T - 1:
                    prep3(0, g + 1)
                s3_frame(0, g, fb[0]); fb[0] += 1
                g += 1
        s2_cc(1)
        drain = 0
        while g < T:
            if g < T - 1:
                prep3(0, g + 1)
            s3_frame(0, g, fb[0]); fb[0] += 1
            g += 1
            drain += 1
            if drain == 2:
                cc_finish(3, M2C, R2C, 1)
                fold_r2(1)
        s3_cc(0)
        prep3(1, 0)
        for g in range(LEAD):
            if g < T - 1:
                prep3(1, g + 1)
            s3_frame(1, g, fb[0]); fb[0] += 1
            if g == 2:
                cc_finish(4, M3C, R3C, 0)
        sef = 0
        for g in range(LEAD, T):
            if g < T - 1:
                prep3(1, g + 1)
            s3_frame(1, g, fb[0]); fb[0] += 1
            if sef < T:
                se_frame(0, sef); sef += 1
        s3_cc(1)
        while sef < T:
            se_frame(0, sef); sef += 1
        # pool CC for blk0 now; it hides under se(1)
        reduce_cols(ccs[:, 12:13], ("pl", 0), B_PL[0])
        nc.sync.dma_start(out=pl_i[0][:, :], in_=ccs[:, 12:13])
        nc.gpsimd.collective_compute(
            "AllGather", OP.bypass, replica_groups=GROUPS,
            ins=[pl_i[0][:, :]], outs=[pl_o[0][:, :]])
        cc_finish(5, M3C, R3C, 1)
        for f in range(T):
            se_frame(1, f)
        reduce_cols(ccs[:, 13:14], ("pl", 1), B_PL[1])
        nc.sync.dma_start(out=pl_i[1][:, :], in_=ccs[:, 13:14])
        nc.gpsimd.collective_compute(
            "AllGather", OP.bypass, replica_groups=GROUPS,
            ins=[pl_i[1][:, :]], outs=[pl_o[1][:, :]])
        r = ccr[6]
        for blk in range(2):
            nc.sync.dma_start(out=r[:, 2 * blk:2 * blk + 1], in_=pl_o[blk][0:128, :])
            nc.sync.dma_start(out=r[:, 2 * blk + 1:2 * blk + 2], in_=pl_o[blk][128:256, :])
            nc.vector.tensor_tensor(
                r[:, 2 * blk:2 * blk + 1], r[:, 2 * blk:2 * blk + 1],
                r[:, 2 * blk + 1:2 * blk + 2], OP.add)
        # pooled_hat[:, blk] = sum * r3 / NPIX
        for blk in range(2):
            b = 16 * blk
            nc.vector.tensor_scalar(ss(TPS), ss(R3C + b), 1.0 / NPIX, None, OP.mult)
            nc.vector.tensor_tensor(
                ss(POOLC + b), r[:, 2 * blk:2 * blk + 1], ss(TPS), OP.mult)
        # SE MLP
        psz = PS[0:64, 3584:3585]
        for blk in range(2):
            nc.tensor.matmul(
                psz, wse1sb[:, blk * 64:(blk + 1) * 64],
                ss(POOLC + 16 * blk), start=(blk == 0), stop=(blk == 1))
        nc.vector.tensor_scalar(zsb[:, :], psz, 0.0, None, OP.max)
        for blk in range(2):
            b = 16 * blk
            psy = PS[:, 3600 + blk:3601 + blk]
            nc.tensor.matmul(
                psy, wse2sb[:, blk * 128:(blk + 1) * 128], zsb[:, :],
                start=True, stop=True)
            # sigmoid via exp + reciprocal (stays in ln/exp table set)
            nc.scalar.activation(ss(TP0 + b), psy, AF.Exp, bias=ss(ZEROC), scale=-1.0)
            nc.vector.tensor_scalar(ss(TP1 + b), ss(TP0 + b), 1.0, None, OP.add)
            nc.vector.reciprocal(ss(TP0 + b), ss(TP1 + b))
            # ya3 = y * r3 ; wp = w_projT * ya3  (bf16)
            nc.vector.tensor_tensor(ss(YA3 + b), ss(TP0 + b), ss(R3C + b), OP.mult)
            nc.vector.tensor_scalar(
                wpb[:, blk * 64:(blk + 1) * 64], wprojsb[:, blk * 64:(blk + 1) * 64],
                ss(YA3 + b), None, OP.mult)

        # ================= proj (in-place into A0) =================
        pr_i = [0]
        for f in range(T):
            for pair, y0 in enumerate((0, 16)):
                off = 512 * (pr_i[0] % 4)
                pr_i[0] += 1
                for half, yh in enumerate((y0, y0 + 8)):
                    for blk in range(2):
                        nc.tensor.matmul(
                            PS[half * 64:half * 64 + 64, off:off + 512],
                            wpb[:, blk * 64:(blk + 1) * 64],
                            A[blk][:, f, yh:yh + 8, :].rearrange("p a b -> p (a b)"),
                            start=(blk == 0), stop=(blk == 1))
                # copy packed pair tile into A0 (consumed region), accum stats
                c = sc_col("s4s", B_S4S)
                dst = A0[:, f, y0:y0 + 8, :].rearrange("p a b -> p (a b)")
                nc.vector.tensor_scalar(dst, PS[:, off:off + 512], 1.0, None, OP.mult,
                                        OP.add, accum_out=SC[:, c:c + 1])
                c = sc_col("s4q", B_S4Q)
                dst3 = A0[:, f, y0:y0 + 8, :]
                nc.gpsimd.tensor_tensor(
                    SCRP[:, 0:512].rearrange("p (a b) -> p a b", b=64), dst3, dst3, OP.mult)
                nc.vector.tensor_scalar(SCRD[:, 0:512], SCRP[:, 0:512], 1.0, None,
                                        OP.mult, OP.add, accum_out=SC[:, c:c + 1])

        # stats4: per-channel sums live on packed partitions; fold 64:128 into 0:64
        reduce_cols(ccs[:, 14:15], "s4s", B_S4S)
        reduce_cols(ccs[:, 15:16], "s4q", B_S4Q)
        nc.sync.dma_start(out=ccs[0:64, 0:2], in_=ccs[64:128, 14:16])
        nc.vector.tensor_tensor(ccs[0:64, 14:16], ccs[0:64, 14:16], ccs[0:64, 0:2], OP.add)
        nc.sync.dma_start(out=c4_i[:, :], in_=ccs[0:64, 14:16])
        nc.gpsimd.collective_compute(
            "AllGather", OP.bypass, replica_groups=GROUPS,
            ins=[c4_i[:, :]], outs=[c4_o[:, :]])
        r = ccr[7]
        nc.sync.dma_start(out=r[0:64, 0:2], in_=c4_o[0:64, :])
        nc.sync.dma_start(out=r[0:64, 2:4], in_=c4_o[64:128, :])
        nc.sync.dma_start(out=r[64:128, 0:2], in_=c4_o[0:64, :])
        nc.sync.dma_start(out=r[64:128, 2:4], in_=c4_o[64:128, :])
        nc.vector.tensor_tensor(r[:, 0:2], r[:, 0:2], r[:, 2:4], OP.add)
        stats_from(r[:, 0:1], r[:, 1:2], M4C - 32, R4C - 32, 32)
        # s1f = -m4*r4 (all 128 partitions: packed layout needs both halves)
        nc.vector.tensor_tensor(ss(TPS), ss(M4C), ss(R4C), OP.mult)
        nc.vector.tensor_scalar(ss(S1F), ss(TPS), -1.0, None, OP.mult)

        # ================= final: affine + residual + maxpool ==============
        for f in range(T):
            xt = xin_pool.tile([128, 1024], BF16, tag="xr")
            nc.sync.dma_start(out=xt[:, :], in_=xres[:, f, :])
            ot = fin_pool.tile([128, 2, 4, 32], BF16, tag="ot")
            for pair, y0 in enumerate((0, 16)):
                p_ap = A0[:, f, y0:y0 + 8, :].rearrange("p a b -> p (a b)")
                tf = fin_pool.tile([128, 8, 64], BF16, tag="tf")
                tf_ap = tf[:, :, :].rearrange("p a b -> p (a b)")
                nc.vector.tensor_scalar(
                    tf_ap, p_ap, ss(R4C), ss(S1F), OP.mult, OP.add)
                nc.gpsimd.tensor_tensor(
                    tf_ap, tf_ap, xt[:, pair * 512:(pair + 1) * 512], OP.add)
                a2 = tf[:, :, :].rearrange("p y (x t) -> p y x t", t=2)
                mp1 = fin_pool.tile([128, 8, 32], BF16, tag="mp1")
                nc.vector.tensor_tensor(mp1[:, :, :], a2[:, :, :, 0], a2[:, :, :, 1], OP.max)
                b2 = mp1[:, :, :].rearrange("p (y t) x -> p y t x", t=2)
                nc.vector.tensor_tensor(
                    ot[:, pair, :, :], b2[:, :, 0, :], b2[:, :, 1, :], OP.max)
            nc.sync.dma_start(
                out=out[:, f, :],
                in_=ot[:, :, :, :].rearrange("p a b c -> p (a b c)"))

    import bass_rust as _br
    _br.move_matmul_waits_to_ldweights(nc.m)
    _br.generate_event_semaphores(nc)
    return nc


_CACHE = {}


def build_in_maps(x, w1, w_dw_s, w_dw_t, w_se1, w_se2, w_proj):
    x = np.ascontiguousarray(x, np.float32)
    B = x.shape[0]

    xpad = np.zeros((B, CIN, T, H + 2, W), np.float32)
    xpad[:, :, :, 1:65, :] = x
    w1t = np.ascontiguousarray(w1.T.astype(ml_dtypes.bfloat16))

    diag8 = np.zeros((128, 16, 128), ml_dtypes.bfloat16)
    idx = np.arange(128)
    wcd = np.zeros((128, 2), np.float32)
    for blk in range(2):
        for k, (dy, dx) in enumerate(TAPS8):
            diag8[idx, blk * 8 + k, idx] = w_dw_s[blk * 128:(blk + 1) * 128, 0, 0, dy, dx].astype(
                ml_dtypes.bfloat16)
        wcd[:, blk] = w_dw_s[blk * 128:(blk + 1) * 128, 0, 0, 1, 1]
    diag8 = np.ascontiguousarray(diag8.reshape(128, 16 * 128))

    wtd = np.zeros((128, 6, 128), ml_dtypes.bfloat16)
    wt1 = np.zeros((128, 2), np.float32)
    for blk in range(2):
        for tap in range(3):
            wtd[idx, blk * 3 + tap, idx] = w_dw_t[blk * 128:(blk + 1) * 128, 0, tap, 0, 0].astype(
                ml_dtypes.bfloat16)
        wt1[:, blk] = w_dw_t[blk * 128:(blk + 1) * 128, 0, 1, 0, 0]
    wtd = np.ascontiguousarray(wtd.reshape(128, 6 * 128))

    wse1t = np.ascontiguousarray(
        np.concatenate([w_se1[:, :128].T, w_se1[:, 128:].T], axis=1), np.float32)
    wse2t = np.ascontiguousarray(w_se2.T, np.float32)
    wprojt = np.ascontiguousarray(
        np.concatenate([w_proj[:, :128].T, w_proj[:, 128:].T], axis=1), np.float32)

    in_maps = []
    for core in range(8):
        b, j = core // 2, core % 2
        hsv = np.ones((128, 2), np.float32)
        if j == 0:
            hsv[:, 0] = 0.0
        else:
            hsv[:, 1] = 0.0
        xo = x[b, :, :, 32 * j:32 * j + 32, :]  # [64, 16, 32, 64]
        xr = np.ascontiguousarray(
            xo.reshape(64, 16, 2, 2, 8, 64).transpose(3, 0, 1, 2, 4, 5)
            .reshape(128, 16, 1024).astype(ml_dtypes.bfloat16))
        in_maps.append({
            "xs16": np.ascontiguousarray(
                xpad[b, :, :, 32 * j:32 * j + 34, :].astype(ml_dtypes.bfloat16)),
            "w1t": w1t,
            "diag8": diag8,
            "wcd": wcd,
            "wtd": wtd,
            "wt1": wt1,
            "wse1t": wse1t,
            "wse2t": wse2t,
            "wprojt": wprojt,
            "hs": hsv,
            "xres": xr,
        })
    return in_maps


def unpack_out(res_out):
    # res_out: [128, 16, 256] bf16 -> [64, 16, 16, 32] fp32
    o = np.asarray(res_out, dtype=np.float32).reshape(2, 64, 16, 2, 4, 32)
    return o.transpose(1, 2, 3, 0, 4, 5).reshape(64, 16, 16, 32)


def kernel(x, w1, w_dw_s, w_dw_t, w_se1, w_se2, w_proj):
    B = x.shape[0]
    if "nc" not in _CACHE:
        _CACHE["nc"] = _build_nc()
    nc = _CACHE["nc"]
    in_maps = build_in_maps(x, w1, w_dw_s, w_dw_t, w_se1, w_se2, w_proj)

    res = run_bass_kernel_spmd(nc, in_maps, core_ids=list(range(8)))
    _CACHE["exec_time_ns"] = getattr(res, "exec_time_ns", None)
    _CACHE["results"] = res.results
    _CACHE["res"] = res
    out = np.zeros((B, CO, T, 32, 32), np.float32)
    for core in range(8):
        b, j = core // 2, core % 2
        out[b, :, :, 16 * j:16 * j + 16, :] = unpack_out(res.results[core]["out"])
    return out
